# revision 35
# baseline (speedup 1.0000x reference)
"""Trainium2 Bass kernel for nn_ExpandEvecs.

Computes, for evecs [B=4, C=1, M=1024, K=32] and max_lvl=16, the stack of
cumulative low-rank reconstructions
    out[b, l] = V[:, :l+1] @ V[:, :l+1]^T      (V = evecs[b, 0, :, :max_lvl])
returned as [B, max_lvl, M, M] float32 (256 MiB full output).

Every level's matrix is SYMMETRIC, so the device only computes/writes the
upper-triangle 128-row blocks: row-block q (rows 128q..128q+127) covers
columns 128q..1023.  That is 36 of 64 blocks (56.25% of the bytes and of
the PE column streaming); assemble() mirrors the lower triangle on the
host (a numpy transpose-copy) and upcasts bf16 -> fp32.

SPMD trick: run_bass_kernel_spmd runs ONE program on all cores, so the
triangle is chopped into <=512-col chunks whose width multiset
{512x6, 384x2, 256x2, 128x2} splits into two IDENTICAL halves
{512x3, 384, 256, 128} = 2304 cols/level/core.  The host packs per-core
lhs/rhs input tensors (slices of vt) so the same static
6-chunks-per-level program computes either half; PIECES records the
chunk -> (row-block, col-range) map for host-side assembly.

PE row tiling: every matmul has contraction r = l+1 <= 16 <= 32, so
matmuls placed at tile_position (0,0) / (32,0) run CONCURRENTLY in the
128x128 array (row-groups; measured ~3x for 4-tile K=32 packs, ~2x for
the 2-way packing used here).  The packed lhs/rhs inputs are replicated
by the host at partition offsets 0/32; chunk i uses row-group GROUPS[i].
CRITICAL: two concurrent matmuls (different row-groups) must never
write the same PSUM bank -- that crashes the device.  Chunks that share
a bank share a row-group (serialize).

Sharding: core c handles batch b = c//2, triangle half c % 2.

Measured facts (traces; full-matrix baseline + this kernel):
- Output is BF16: fp16 passes the gate at 3.3e-4 norm error but ScalarE
  pays 1.44ns/elem for the fp32->fp16 ROUNDING in its copies; bf16
  truncation runs at 1.09ns/elem and its quantization (1.7e-3) is still
  12x under the 2e-2 gate.  fp8 output measured 2.3e-2 (all levels) /
  1.8e-2 (levels 12+ only): dead end.
- ALL dmas ride the sync-hosted HWDGE ring (queue 1).  Dual-ring output
  provokes a ring-host (DMA engine 15) head-of-line collision; the ring
  host also runs ~6-20% slow, trailing the stream end by ~1.5us.
- Every output dma_start covers all 128 partitions: descriptor count n
  stripes packets over E = largest divisor of n <= 16 engines; n=120
  (E=15) measured 212-309 GB/s vs 400-442 GB/s for n=128.  Level pairs
  (9216B descriptors) beat per-level singles slightly.
- Issuing a dma_start costs the HOSTING engine ~600ns: input dmas on
  the scalar ring delay its first copies; many input dmas ahead of the
  outputs on the FIFO sync ring delay the whole stream.  Hence exactly
  4 input dma_starts (2-way replica x prefix/rest split).
- Per-core DMA data bandwidth peaks ~440 GB/s; output is 9 MiB/core ->
  ~23.5 us of stream, plus ~8.5us framework preamble, ~3.5us
  input+level-0 ramp, ~2.8us teardown.
- PSUM -> SBUF copies (with the fp32->fp16 cast) can only run on
  VectorE (measured 1.19 ns/elem + ~160ns/instr) and ScalarE (measured
  1.44 ns/elem + cheap-ish small copies); GPSIMD has no PSUM port.
  Copies merge to 3 per level (1024 on V, 1024 on S, 256 balanced
  V-11/S-5 levels) via 2-bank PSUM tiles: one pool tag, 4 bufs x 4KB =
  exactly 8 banks.  Producer span (~24.5us/engine) ~= ring drain rate:
  both are the roofline.  Tail levels 14/15 DMA per level so only ~1.3us
  of stream trails the last copy.
- PE p-state: cold 1.54 ns/col, mid 0.83; two dummy matmuls at kernel
  start warm it before level 0.
"""

import sys

for _p in ("/root/.axon_site/_ro/trn_rl_repo", "/opt/trn_rl_repo"):
    if _p not in sys.path:
        sys.path.insert(0, _p)

import numpy as np

import concourse.bacc as bacc
import concourse.mybir as mybir
from concourse.tile import TileContext
from concourse import bass_utils

B, C, M, K, L = 4, 1, 1024, 32, 16
P = 128
F32 = mybir.dt.float32
F16 = mybir.dt.float16
BF16 = mybir.dt.bfloat16

# Static per-core chunk widths (identical on every core), and per-half
# (row-block q, col-offset-within-block cc) for each chunk.  Chunk i of
# half h computes out rows [128q, 128q+128) x cols [128q+cc, 128q+cc+w).
CHUNK_W = (512, 512, 512, 384, 128, 256)
W = sum(CHUNK_W)  # 2304 columns per level per core
PIECES = {
    0: ((0, 0), (1, 0), (2, 0), (5, 0), (7, 0), (6, 0)),
    1: ((0, 512), (3, 0), (4, 0), (1, 512), (3, 512), (2, 512)),
}
NCH = len(CHUNK_W)
GROUPS = (0, 1, 0, 1, 1, 0)  # PE row-group per chunk; chunks 3,4 share a
# PSUM bank and therefore a row-group (same-group matmuls serialize -- two
# CONCURRENT matmuls must never write the same PSUM bank)

OUT_BUFS = 4
TILE4 = True


def build_nc(out_bufs=None, tile4=None):
    if out_bufs is None:
        out_bufs = OUT_BUFS
    if tile4 is None:
        tile4 = TILE4
    nc = bacc.Bacc("TRN2", target_bir_lowering=False, debug=False)
    # rep: packed [lhs | rhs], replicated at partition offsets 0/32 (the
    # two PE row-groups), prefix rows 0:4 DMA'd first so levels 0-3 start
    # as soon as ~25KB lands.
    C = NCH * P + W
    rep_d = nc.dram_tensor("rep", [2, L, C], F16, kind="ExternalInput")
    out = nc.dram_tensor("out", [P, L * W], BF16, kind="ExternalOutput")
    out_v = out.ap().rearrange("p (l w) -> p l w", w=W)

    # static chunk offsets within a level's packed output row
    offs = [0]
    for w in CHUNK_W:
        offs.append(offs[-1] + w)

    with TileContext(nc) as tc:
        with (
            tc.tile_pool(name="consts", bufs=1) as consts,
            tc.tile_pool(name="outp", bufs=out_bufs) as outp,
            tc.tile_pool(name="psum", bufs=4, space="PSUM") as psump,
        ):
            # Packed inputs replicated at partition offsets 0/32 (the two
            # PE row-groups), prefix rows 0:4 first so levels 0-3 only wait
            # for ~25KB.  All four dma_starts ride the sync ring ahead of
            # the outputs; the scalar engine must stay free for copies.
            # group 0 on the scalar ring, group 1 on the sync ring: the two
            # prefixes land in parallel ~0.6us earlier, and only two input
            # dma_starts sit ahead of the outputs on the sync ring's FIFO.
            rep = consts.tile([64, C], F16)
            for g in range(2):
                eng = nc.scalar if g == 0 else nc.sync
                eng.dma_start(out=rep[32 * g : 32 * g + 4, :], in_=rep_d.ap()[g][0:4])
            for g in range(2):
                eng = nc.scalar if g == 0 else nc.sync
                eng.dma_start(
                    out=rep[32 * g + 4 : 32 * g + L, :], in_=rep_d.ap()[g][4:L]
                )

            # PE warmup: the tensor engine starts at cold p-state
            # (1.54 ns/col); two dummy matmuls overlap the input DMA latency
            # so level 0 runs at mid p-state.
            wt = consts.tile([16, 640], F16)
            nc.vector.memset(wt[0:16, :], 0.0)
            wp = psump.tile([P, 1024], F32, name="ps")
            for _ in range(2):
                nc.tensor.matmul(
                    wp[:, 0:512],
                    wt[0:16, 0:128],
                    wt[0:16, 128:640],
                    start=True,
                    stop=True,
                )

            # Levels 0,1 DMA per level to ramp the stream; levels 2+ DMA
            # per level-pair (9216B descriptors).
            ot = None
            for l in range(L):
                r = l + 1
                if l in (0, 1):
                    s = 0
                    ot = outp.tile([P, W], BF16, name="ot_single")
                else:
                    s = l % 2
                    if s == 0:
                        ot = outp.tile([P, 2 * W], BF16, name="ot_pair")
                so = s * W
                # three PSUM tiles per level; the level's chunks land at
                # [a1 | a1 | a2 | a2 | a2 | b] so copies merge to 3.
                a1 = psump.tile([P, 1024], F32, name="ps")
                a2 = psump.tile([P, 1024], F32, name="ps")
                bb = psump.tile([P, 1024], F32, name="ps")
                dsts = (
                    a1[:, 0:512],
                    a1[:, 512:1024],
                    a2[:, 0:512],
                    a2[:, 512:896],
                    a2[:, 896:1024],
                    bb[:, 0:256],
                )
                for i, wch in enumerate(CHUNK_W):
                    g = GROUPS[i] if tile4 else 0
                    p0 = 32 * g
                    nc.tensor.matmul(
                        dsts[i],
                        rep[p0 : p0 + r, i * P : (i + 1) * P],
                        rep[p0 : p0 + r, NCH * P + offs[i] : NCH * P + offs[i] + wch],
                        start=True,
                        stop=True,
                        tile_position=(p0, 0) if tile4 else None,
                    )
                if l == 0:
                    # ramp: split the V copy per chunk so the first 512-col
                    # piece streams ~1.3us earlier on the idle ring
                    nc.vector.tensor_copy(out=ot[:, 0:512], in_=a1[:, 0:512])
                    nc.vector.tensor_copy(out=ot[:, 512:1024], in_=a1[:, 512:1024])
                else:
                    nc.vector.tensor_copy(out=ot[:, so : so + 1024], in_=a1[:, :])
                nc.scalar.copy(out=ot[:, so + 1024 : so + 2048], in_=a2[:, :])
                # With bf16 output ScalarE copies at ~1.09ns/elem (fp16
                # rounding cost 1.44!) vs VectorE 1.19; marginal small-copy
                # cost is ~260ns (V) / ~300ns (S).  Spans balance with the
                # small copy on V for 8 of 16 levels (~23.2us each engine).
                if 5 <= l <= 12:
                    nc.vector.tensor_copy(
                        out=ot[:, so + 2048 : so + 2304], in_=bb[:, 0:256]
                    )
                else:
                    nc.scalar.copy(out=ot[:, so + 2048 : so + 2304], in_=bb[:, 0:256])
                if l == 0:
                    ovl = out_v[:, l, :].rearrange("p (x w) -> p x w", x=1)
                    nc.sync.dma_start(
                        out=ovl[:, :, 0:512],
                        in_=ot[:, 0:512].rearrange("p (x w) -> p x w", x=1),
                    )
                    nc.sync.dma_start(
                        out=ovl[:, :, 512:1024],
                        in_=ot[:, 512:1024].rearrange("p (x w) -> p x w", x=1),
                    )
                    nc.sync.dma_start(
                        out=ovl[:, :, 1024:W],
                        in_=ot[:, 1024:W].rearrange("p (x w) -> p x w", x=1),
                    )
                elif l == 1:
                    nc.sync.dma_start(
                        out=out_v[:, l : l + 1, :],
                        in_=ot[:, :].rearrange("p (x w) -> p x w", x=1),
                    )
                elif l >= L - 2:
                    # tail: per-level singles halve the post-last-copy drain
                    nc.sync.dma_start(
                        out=out_v[:, l : l + 1, :],
                        in_=ot[:, so : so + W].rearrange("p (x w) -> p x w", x=1),
                    )
                elif s == 1:
                    nc.sync.dma_start(
                        out=out_v[:, l - 1 : l + 1, :],
                        in_=ot[:, :].rearrange("p (l w) -> p l w", w=W),
                    )
    nc.compile()
    return nc


_NC_CACHE = {}


def _get_nc():
    key = (OUT_BUFS, TILE4)
    if key not in _NC_CACHE:
        _NC_CACHE[key] = build_nc()
    return _NC_CACHE[key]


def make_in_maps(evecs):
    evecs = np.asarray(evecs, dtype=np.float32)
    in_maps = []
    for core in range(8):
        b, half = core // 2, core % 2
        vt = np.ascontiguousarray(evecs[b, 0, :, :L].T).astype(np.float16)
        lhs = np.empty((L, NCH * P), dtype=np.float16)
        rhs = np.empty((L, W), dtype=np.float16)
        off = 0
        for i, (q, cc) in enumerate(PIECES[half]):
            w = CHUNK_W[i]
            lhs[:, i * P : (i + 1) * P] = vt[:, 128 * q : 128 * (q + 1)]
            rhs[:, off : off + w] = vt[:, 128 * q + cc : 128 * q + cc + w]
            off += w
        cat = np.concatenate([lhs, rhs], axis=1)  # [L, C]
        rep = np.ascontiguousarray(
            np.broadcast_to(cat, (2, L, cat.shape[1])), dtype=np.float16
        )
        in_maps.append({"rep": rep})
    return in_maps


def assemble(results):
    fullh = np.empty((B, L * C, M, M), dtype=np.float16)
    for core in range(8):
        b, half = core // 2, core % 2
        arr = results[core]["out"].astype(np.float16).reshape(P, L, W)
        off = 0
        for i, (q, cc) in enumerate(PIECES[half]):
            w = CHUNK_W[i]
            c0 = 128 * q + cc
            fullh[b, :, 128 * q : 128 * (q + 1), c0 : c0 + w] = arr[
                :, :, off : off + w
            ].transpose(1, 0, 2)
            off += w
    # mirror the lower triangle (every level's matrix is symmetric)
    for i in range(8):
        si = slice(128 * i, 128 * (i + 1))
        for j in range(i + 1, 8):
            sj = slice(128 * j, 128 * (j + 1))
            fullh[:, :, sj, si] = fullh[:, :, si, sj].swapaxes(-1, -2)
    return fullh.astype(np.float32)


def kernel(evecs, max_lvl):
    assert int(max_lvl) == L, f"kernel hardcodes max_lvl={L}, got {max_lvl}"
    nc = _get_nc()
    res = bass_utils.run_bass_kernel_spmd(nc, make_in_maps(evecs), list(range(8)))
    return assemble(res.results)
